# revision 37
# baseline (speedup 1.0000x reference)
"""Multi-head attention (B=2, N=2304, C=768, 12 heads) on 8 Trainium2 cores.

Sharding: tensor-parallel over (batch, heads). Core i handles batch b=i//4
and heads 3*(i%4) .. 3*(i%4)+2. Each core computes a partial projection
output [2304, 768] (bf16); the host sums the 4 partials of each batch
group in fp32 and adds proj_b.

Key TRN2 facts this kernel exploits (measured on this part):
  - PE streams 1 col/cycle at 2.4GHz; two K=64 matmuls at tile_position
    (0,0)/(64,0) dual-issue on the two halves of the PE array (~2x) ->
    S^T pairs use duplicated per-head Q/K halves. PSUM dsts of a dual
    pair must sit in different PSUM banks.
  - The exp of S is the serial engine bottleneck of attention, so it is
    SPLIT across two engines: ACT computes exact exp (1 elem/cyc/lane
    @1.2GHz) for 5 of every 9 j-chunk pairs; DVE computes a Schraudolph
    int16-bitcast approximate exp out = bitcast_bf16(int16(round(
    s*a+b))) (max rel err ~3.3%) via a single tensor_scalar (mult,add)
    for the other 4. Verified end-to-end rel err ~1.25e-2 (< 2e-2).
  - DVE ops with an fp32 PSUM source always run at 1 elem/cycle (one
    32-bit PSUM read port), which sets the DVE exp/cast budget.
  - Everything after qkv is fused in one software-pipelined loop over
    (i-block, head, j-chunk-pair) steps: S dual-pair -> exp (ACT|DVE,
    same batch) -> [O^T; denom] accumulation (lags one batch); the norm
    (1/denom = exp(-ln(denom)) on ACT, PE ones-broadcast, DVE mul) lags
    behind; the projection of a finished i-block (uniform K=128 via
    zero-padded O2/wp2) rides inside the next block's engine-bound
    window. The 256-wide leftover i-block goes FIRST (its low-intensity
    steps land in the pipeline ramp) packed two j-pairs per PSUM tile.
  - PSUM budget (8 banks): s_ps pool 3x[128,1024] (S pairs + proj +
    norm-broadcast share it) + o_ps 2x[65,512] = exactly 16KB/partition.

Device dataflow (per core):
  phase 1 : qT/kT = wqkT.T @ xT (3 merged M=128 chains), dup to both
            partition halves (PSUM->SBUF copies on DVE + ACT); V natural
            via xT-chunk-stationary matmuls, with a ones column per
            j-chunk for the softmax denominators
  fused   : per i-block, per head: S^T dual pairs -> exp -> O^T/denom;
            norm lags; proj + cast (ACT/DVE alternating) + DMA out lag
            one block
"""

import sys

for _p in ("/opt/trn_rl_repo",):
    if _p not in sys.path:
        sys.path.insert(0, _p)

import numpy as np
import ml_dtypes

import concourse.bass as bass
import concourse.mybir as mybir
import concourse.tile as tile
from concourse.bass_utils import run_bass_kernel_spmd

F32 = mybir.dt.float32
F32R = mybir.dt.float32r
BF16 = mybir.dt.bfloat16
I16 = mybir.dt.int16
EXP = mybir.ActivationFunctionType.Exp
LN = mybir.ActivationFunctionType.Ln

DIM = 768
HEADS = 12
D = 64
SEQ = 2304
BATCH = 2
HC = 3  # heads per core
SCALE = D ** (-0.5)
NBLK = [(2048, 256), (0, 512), (512, 512), (1024, 512), (1536, 512)]
NJ = SEQ // 128  # 18 j-chunks
NJP = NJ // 2  # 9 dual-issued j-chunk pairs
NCCHUNK = DIM // 128  # 6 contraction chunks

# Schraudolph exp constants (bf16 via int16 bit trick), calibrated for
# round-to-nearest fp32->int16 conversion (verified on HW).
A_EXP = float(np.float32(128.0 / np.log(2.0) * SCALE))
B_EXP = 16250.5

CTRL_TYPES = ("InstDrain", "InstNoOp", "InstEventSemaphore", "InstSemClear")


def _split_waits(nc, max_waits=1, compute_max=None):
    """This container's walrus accepts only one sync-wait per CTRL-type
    instruction; Tile emits several (notably on the kernel-tail drain).
    Move extras onto same-engine NoOps inserted immediately before."""
    n_new = 0
    for f in nc.m.functions:
        for b in f.blocks:
            il = b.instructions
            i = 0
            while i < len(il):
                inst = il[i]
                lim = max_waits
                if compute_max is not None and type(inst).__name__ not in CTRL_TYPES:
                    lim = compute_max
                si = inst.sync_info
                waits = list(si.on_wait) if (si and si.on_wait) else []
                if len(waits) > lim:
                    extra, keep = waits[:-lim], waits[-lim:]
                    k = 0
                    while extra:
                        chunk, extra = extra[:1], extra[1:]
                        nop = mybir.InstNoOp(
                            name=f"{inst.name}-wsplit-{k}",
                            engine=inst.engine,
                            sync_info=mybir.SyncInfo(on_wait=chunk, on_update=[]),
                        )
                        nc.register_instruction(nop, overwrite=True)
                        il.insert(i, nop)
                        i += 1
                        n_new += 1
                        k += 1
                    inst.sync_info = mybir.SyncInfo(
                        on_wait=keep,
                        on_update=list(si.on_update) if si.on_update else [],
                    )
                i += 1
    return n_new


def build_program():
    import os

    # exp engine assignment over the 9 j-chunk pairs: ACT = exact exp,
    # DVE = direct Schraudolph from PSUM, GPS = DVE 2x-cast + GpSimd
    # Schraudolph from SBUF (three engines run concurrently).
    dve_jp = {int(c) for c in os.environ.get("DVEJP", "1357")}
    gps_jp = {int(c) for c in os.environ.get("GPSJP", "")}

    nc = bass.Bass()
    xT = nc.declare_dram_parameter("xT", [DIM, SEQ], BF16, isOutput=False)
    wqkT = nc.declare_dram_parameter("wqkT", [DIM, 6 * D], BF16, isOutput=False)
    wvT = nc.declare_dram_parameter("wvT", [DIM, HC * D], BF16, isOutput=False)
    wpT = nc.declare_dram_parameter("wpT", [HC * D, DIM], BF16, isOutput=False)
    out = nc.declare_dram_parameter("out_part", [SEQ, DIM], BF16, isOutput=True)

    with tile.TileContext(nc) as tc:
        with tc.tile_pool(name="sb", bufs=1) as sb:
            wpool = qpool = ptpool = opool = spool = ostpool = sb
            # ---- persistent q/k (dup'ed halves), V natural, O tiles ----
            Tq_t = qpool.tile([128, HC * SEQ], BF16, name="Tq", tag="Tq")
            Tk_t = qpool.tile([128, HC * SEQ], BF16, name="Tk", tag="Tk")
            Tq = [Tq_t[:, h * SEQ : (h + 1) * SEQ] for h in range(HC)]
            Tk = [Tk_t[:, h * SEQ : (h + 1) * SEQ] for h in range(HC)]
            # Per-head V slices; 66-wide chunk groups [v(64)|1|pad] keep
            # every lhsT slice at an even bf16 element offset.
            V_t = qpool.tile([128, HC * NJ * 66], BF16, name="V", tag="V")
            V = [V_t[:, h * NJ * 66 : (h + 1) * NJ * 66] for h in range(HC)]
            nc.gpsimd.memset(V_t[:], 1.0)
            O01c = opool.tile([128, SEQ], BF16, name="O01c", tag="O01c")
            # O2/wp2 zero-padded to K=128 so proj matmuls are shape-uniform
            O2p = opool.tile([128, SEQ], BF16, name="O2p", tag="O2p")
            nc.gpsimd.memset(O2p[:], 0.0)
            ones1f = wpool.tile([1, 64], F32, name="ones1f", tag="ones1f")
            nc.gpsimd.memset(ones1f[:], 1.0)
            ones1 = wpool.tile([1, 64], F32R, name="ones1", tag="ones1")
            nc.vector.tensor_copy(ones1[:], ones1f[:])

            # ---- weights; single big tiles (fewer tags -> fewer sems);
            # (wqk c, x block-0 c) DMA pairs first so chain 0 starts early
            wqk_t = wpool.tile([128, NCCHUNK * 6 * D], BF16, name="wqk", tag="wqk")
            xb_t = wpool.tile([128, NCCHUNK * SEQ], BF16, name="xb", tag="xb")
            wv_t = wpool.tile([128, NCCHUNK * HC * D], BF16, name="wv", tag="wv")
            wqk = [wqk_t[:, c * 6 * D : (c + 1) * 6 * D] for c in range(NCCHUNK)]
            xb = [xb_t[:, c * SEQ : (c + 1) * SEQ] for c in range(NCCHUNK)]
            wv = [wv_t[:, c * HC * D : (c + 1) * HC * D] for c in range(NCCHUNK)]
            n0, nsz = NBLK[0]
            for c in range(NCCHUNK):
                nc.sync.dma_start(wqk[c], wqkT[c * 128 : (c + 1) * 128, :])
                nc.sync.dma_start(
                    xb[c][:, n0 : n0 + nsz], xT[c * 128 : (c + 1) * 128, n0 : n0 + nsz]
                )
            for c in range(NCCHUNK):
                nc.sync.dma_start(wv[c], wvT[c * 128 : (c + 1) * 128, :])
            for lo, hi in ((0, 1024), (1024, 2048)):
                for c in range(NCCHUNK):
                    nc.sync.dma_start(
                        xb[c][:, lo:hi], xT[c * 128 : (c + 1) * 128, lo:hi]
                    )
            wp01 = wpool.tile([128, DIM], BF16, name="wp01", tag="wp01")
            nc.sync.dma_start(wp01[:], wpT[0:128, :])
            wp2p = wpool.tile([128, DIM], BF16, name="wp2p", tag="wp2p")
            nc.gpsimd.memset(wp2p[64:128, :], 0.0)
            nc.sync.dma_start(wp2p[0:64, :], wpT[128:192, :])

            # ---- phase 1: q/k chains + V natural ----
            # wqkT cols: [q0|q1](0:128) [k0|k1](128:256) [q2|k2](256:384)
            with tc.tile_pool(name="pp1", bufs=4, space="PSUM") as ps1:
                psv = ps1
                def emit_chain(n0, nsz, col0, dlo, dhi):
                    xt = [xb[c][:, n0 : n0 + nsz] for c in range(NCCHUNK)]
                    sl = slice(n0, n0 + nsz)
                    ps = ps1.tile([128, nsz], F32, name="ps1", tag="ps1")
                    for c in range(NCCHUNK):
                        nc.tensor.matmul(
                            ps[:],
                            lhsT=wqk[c][:, col0 : col0 + 128],
                            rhs=xt[c],
                            start=(c == 0),
                            stop=(c == NCCHUNK - 1),
                        )
                    # naturals: one per engine (PSUM 1x path); dups: cheap
                    # SBUF->SBUF bf16 copies (DVE 4x mode) off the naturals
                    nc.vector.tensor_copy(dlo[0:64, sl], ps[0:64, :])
                    nc.scalar.copy(dhi[64:128, sl], ps[64:128, :])
                    nc.vector.tensor_copy(dlo[64:128, sl], dlo[0:64, sl])
                    nc.vector.tensor_copy(dhi[0:64, sl], dhi[64:128, sl])

                def emit_chain_pair(n0, nsz, specA, specB):
                    # two chains c-major interleaved: consecutive PE matmuls
                    # alternate PSUM banks (third chain of each block stays
                    # serial so its copies are not delayed)
                    xt = [xb[c][:, n0 : n0 + nsz] for c in range(NCCHUNK)]
                    sl = slice(n0, n0 + nsz)
                    psA = ps1.tile([128, nsz], F32, name="ps1", tag="ps1")
                    psB = ps1.tile([128, nsz], F32, name="ps1", tag="ps1")
                    for c in range(NCCHUNK):
                        for (col0, _, _), ps in ((specA, psA), (specB, psB)):
                            nc.tensor.matmul(
                                ps[:],
                                lhsT=wqk[c][:, col0 : col0 + 128],
                                rhs=xt[c],
                                start=(c == 0),
                                stop=(c == NCCHUNK - 1),
                            )
                    for (col0, dlo, dhi), ps in ((specA, psA), (specB, psB)):
                        nc.vector.tensor_copy(dlo[0:64, sl], ps[0:64, :])
                        nc.scalar.copy(dhi[64:128, sl], ps[64:128, :])
                        nc.vector.tensor_copy(dlo[64:128, sl], dlo[0:64, sl])
                        nc.vector.tensor_copy(dhi[0:64, sl], dhi[64:128, sl])

                for bi, (n0, nsz) in enumerate(NBLK):
                    emit_chain_pair(
                        n0, nsz, (128, Tk[0], Tk[1]), (256, Tq[2], Tk[2])
                    )
                    emit_chain(n0, nsz, 0, Tq[0], Tq[1])
                    # V natural: per j-chunk inside this block
                    for jl in range(nsz // 128):
                        jc = n0 // 128 + jl
                        pv = psv.tile([128, HC * D], F32, name="pv", tag="pv")
                        for c in range(NCCHUNK):
                            nc.tensor.matmul(
                                pv[:],
                                lhsT=xb[c][:, jc * 128 : (jc + 1) * 128],
                                rhs=wv[c][:],
                                start=(c == 0),
                                stop=(c == NCCHUNK - 1),
                            )
                        for h in range(HC):
                            eng = nc.vector.tensor_copy if (jc + h) % 2 else nc.scalar.copy
                            eng(
                                V[h][:, jc * 66 : jc * 66 + 64],
                                pv[:, h * 64 : (h + 1) * 64],
                            )


            # ---- fused attention + norm + proj ----
            with tc.tile_pool(name="pa", bufs=3, space="PSUM") as sps:
                ops = sps
                norm_dst = [
                    lambda s: O01c[0:64, s],
                    lambda s: O01c[64:128, s],
                    lambda s: O2p[0:64, s],
                ]
                o_tiles = {}

                def emit_S(h, i0, isz, jps):
                    # each jp writes its dual pair at [k*isz] / [512+k*isz]
                    # (different banks); a 256-wide block packs 2 jp per tile
                    s_ps = sps.tile([128, 1024], F32, name="s_ps", tag="s_ps")
                    for k, jp in enumerate(jps):
                        jcA, jcB = 2 * jp, 2 * jp + 1
                        nc.tensor.matmul(
                            s_ps[:, k * isz : (k + 1) * isz],
                            lhsT=Tk[h][0:64, jcA * 128 : (jcA + 1) * 128],
                            rhs=Tq[h][0:64, i0 : i0 + isz],
                            start=True,
                            stop=True,
                            tile_position=(0, 0),
                        )
                        nc.tensor.matmul(
                            s_ps[:, 512 + k * isz : 512 + (k + 1) * isz],
                            lhsT=Tk[h][64:128, jcB * 128 : (jcB + 1) * 128],
                            rhs=Tq[h][64:128, i0 : i0 + isz],
                            start=True,
                            stop=True,
                            tile_position=(64, 0),
                        )
                    return s_ps

                def emit_exp(h, i0, isz, jps, s_ps):
                    pt = ptpool.tile([128, 1024], BF16, name="pt", tag="pt", bufs=4)
                    if len(jps) == 1 and isz == 256:
                        # lone half-width unit: two strided segments
                        src = s_ps[:].rearrange("p (g c) -> p g c", g=2)[:, :, 0:isz]
                        dst = pt[:, 0 : 2 * isz].rearrange("p (g c) -> p g c", g=2)
                    else:
                        n = 2 * len(jps) * isz
                        src, dst = s_ps[:, 0:n], pt[:, 0:n]
                    dve = (jps[0] // 2) % 2 == 1 if len(jps) > 1 else jps[0] in dve_jp
                    if dve:
                        nc.vector.tensor_scalar(
                            dst.bitcast(I16),
                            src,
                            A_EXP,
                            B_EXP,
                            mybir.AluOpType.mult,
                            mybir.AluOpType.add,
                        )
                    else:
                        nc.scalar.activation(dst, src, EXP, scale=SCALE)
                    return pt

                def emit_O(h, i0, isz, jps, pt):
                    key = (h, i0)
                    if key not in o_tiles:
                        o_tiles[key] = ops.tile([65, 512], F32, name="o_ps", tag="o_ps", bufs=2)
                    H = len(jps) * isz
                    for k, jp in enumerate(jps):
                        for u in (0, 1):
                            jc = 2 * jp + u
                            nc.tensor.matmul(
                                o_tiles[key][:, 0:isz],
                                lhsT=V[h][:, jc * 66 : jc * 66 + 65],
                                rhs=pt[:, u * H + k * isz : u * H + (k + 1) * isz],
                                start=(jc == 0),
                                stop=(jc == NJ - 1),
                            )

                def emit_norm_recip(h, i0, isz):
                    """1/denom = exp(-ln(denom)) on ACT (table ops, runs in
                    ACT slots between exp tiles)."""
                    o_ps = o_tiles[(h, i0)]
                    lnd = spool.tile([1, 512], F32, name="lnd", tag="lnd", bufs=6)
                    nc.scalar.activation(lnd[:, 0:isz], o_ps[64:65, 0:isz], LN)
                    rec = spool.tile([1, 512], F32R, name="rec", tag="rec", bufs=6)
                    nc.scalar.activation(rec[:, 0:isz], lnd[:, 0:isz], EXP, scale=-1.0)
                    return rec

                def emit_norm_apply(h, i0, isz, rec):
                    o_ps = o_tiles.pop((h, i0))
                    bc_t = sps.tile([128, 1024], F32, name="bc_ps", tag="s_ps")
                    nc.tensor.matmul(
                        bc_t[0:64, 0:isz],
                        lhsT=ones1[:],
                        rhs=rec[:, 0:isz],
                        start=True,
                        stop=True,
                    )
                    rec64 = spool.tile([64, 512], BF16, name="rec64", tag="rec64", bufs=6)
                    nc.vector.tensor_copy(rec64[:, 0:isz], bc_t[0:64, 0:isz])
                    nc.vector.tensor_mul(
                        norm_dst[h](slice(i0, i0 + isz)),
                        o_ps[0:64, 0:isz],
                        rec64[:, 0:isz],
                    )

                def emit_proj(ic):
                    """One i-chunk [128 rows] of the projection: uniform
                    K=128 lhsT chunks (O2p/wp2p zero-padded), 2 LDWs. The
                    768 output cols live in one s_ps-pool tile (2 banks)."""
                    csl = slice(ic * 128, (ic + 1) * 128)
                    p = sps.tile([128, 1024], F32, name="proj", tag="s_ps")
                    nc.tensor.matmul(
                        p[:, 0:512], lhsT=O01c[:, csl], rhs=wp01[:, 0:512],
                        start=True, stop=False,
                    )
                    nc.tensor.matmul(
                        p[:, 512:768], lhsT=O01c[:, csl], rhs=wp01[:, 512:768],
                        start=True, stop=False,
                    )
                    nc.tensor.matmul(
                        p[:, 0:512], lhsT=O2p[:, csl], rhs=wp2p[:, 0:512],
                        start=False, stop=True,
                    )
                    nc.tensor.matmul(
                        p[:, 512:768], lhsT=O2p[:, csl], rhs=wp2p[:, 512:768],
                        start=False, stop=True,
                    )
                    ob = ostpool.tile([128, DIM], BF16, name="ob", tag="ob", bufs=4)
                    if ic % 2 == 0:
                        nc.vector.tensor_copy(ob[:], p[:, 0:768])
                    else:
                        nc.scalar.copy(ob[:], p[:, 0:768])
                    nc.sync.dma_start(out[csl, :], ob[:])

                # ---- software pipeline over (i-block, head, jp) steps ----
                NORM_LAG = 2
                steps = []
                for bi, (n0, nsz) in enumerate(NBLK):
                    units = (
                        [(0, 1), (2, 3), (4, 5), (6, 7), (8,)]
                        if nsz == 256
                        else [(jp,) for jp in range(NJP)]
                    )
                    for h in range(HC):
                        for jps in units:
                            steps.append((bi, h, tuple(jps)))
                recip_q = []  # (h,i0,isz) O done, recip pending
                apply_q = []  # [age, (h,i0,isz,rec)]
                proj_q = []   # i-chunk indices ready to project
                done_h = {}   # bi -> #heads normalized

                def service_norms(flush=False):
                    while recip_q:
                        n = recip_q.pop(0)
                        apply_q.append([0, (*n, emit_norm_recip(*n))])
                    for e in apply_q:
                        e[0] += 1
                    while apply_q and (flush or apply_q[0][0] > NORM_LAG):
                        args = apply_q.pop(0)[1]
                        emit_norm_apply(*args)
                        h, i0, isz = args[0], args[1], args[2]
                        bi = [b for b, (n0, _) in enumerate(NBLK) if n0 == i0][0]
                        done_h[bi] = done_h.get(bi, 0) + 1
                        if done_h[bi] == HC:
                            for k in range(isz // 128):
                                proj_q.append(i0 // 128 + k)

                BSZ = 2
                prev = []
                for bstart in range(0, len(steps), BSZ):
                    batch = steps[bstart : bstart + BSZ]
                    sb = []
                    for bi, h, jps in batch:
                        i0, isz = NBLK[bi]
                        sb.append((h, i0, isz, jps, emit_S(h, i0, isz, jps)))
                    cur = [(*e[:4], emit_exp(*e)) for e in sb]
                    for o in prev:
                        emit_O(*o)
                        if o[3][-1] == NJP - 1:
                            recip_q.append(o[:3])
                    prev = cur
                    service_norms()
                    if proj_q:
                        emit_proj(proj_q.pop(0))
                for o in prev:
                    emit_O(*o)
                    if o[3][-1] == NJP - 1:
                        recip_q.append(o[:3])
                service_norms(flush=True)
                while proj_q:
                    emit_proj(proj_q.pop(0))

    _split_waits(nc, max_waits=1)
    return nc


def make_in_maps(x, qkv_w, proj_w):
    """Per-core host-side sharding: transposed weight slices + x[b].T."""
    x = np.asarray(x, dtype=np.float32)
    qkv_w = np.asarray(qkv_w, dtype=np.float32)
    proj_w = np.asarray(proj_w, dtype=np.float32)
    BF = ml_dtypes.bfloat16
    in_maps = []
    for core in range(8):
        b = core // 4
        h0 = HC * (core % 4)
        q = qkv_w[h0 * D : h0 * D + HC * D, :]        # [192, 768]
        k = qkv_w[DIM + h0 * D : DIM + h0 * D + HC * D, :]
        v = qkv_w[2 * DIM + h0 * D : 2 * DIM + h0 * D + HC * D, :]
        # chains: [q0|q1](128) [k0|k1](128) [q2|k2](128)
        stack = np.concatenate([q[0:128], k[0:128], q[128:192], k[128:192]], axis=0)
        wqkT = np.ascontiguousarray(stack.T).astype(BF)
        wvT_ = np.ascontiguousarray(v.T).astype(BF)
        wpT = np.ascontiguousarray(proj_w[:, h0 * D : (h0 + HC) * D].T).astype(BF)
        xT_ = np.ascontiguousarray(x[b].T).astype(BF)
        in_maps.append({"xT": xT_, "wqkT": wqkT, "wvT": wvT_, "wpT": wpT})
    return in_maps


_PROGRAM_CACHE = {}


def kernel(x, H, W, qkv_w, proj_w, proj_b, **_unused):
    if "nc" not in _PROGRAM_CACHE:
        _PROGRAM_CACHE["nc"] = build_program()
    nc = _PROGRAM_CACHE["nc"]
    in_maps = make_in_maps(x, qkv_w, proj_w)
    res = run_bass_kernel_spmd(nc, in_maps, core_ids=list(range(8)))
    proj_b = np.asarray(proj_b, dtype=np.float32)
    out = np.empty((BATCH, SEQ, DIM), dtype=np.float32)
    for b in range(BATCH):
        acc = res.results[4 * b]["out_part"].astype(np.float32)
        for g in range(1, 4):
            acc = acc + res.results[4 * b + g]["out_part"].astype(np.float32)
        out[b] = acc + proj_b[None, :]
    return out


if __name__ == "__main__":
    nc = build_program()
    n_inst = sum(len(b.instructions) for f in nc.m.functions for b in f.blocks)
    print(f"program built: {n_inst} instructions")
